# revision 15
# baseline (speedup 1.0000x reference)
"""Trainium2 Bass kernel for nn_BaseLineModel (hierarchical sentence->doc model).

v2: fp8 DoubleRow conv + interleaved LSTM.

Per core (4 docs x 32 positions, position-major groups of 4 sentences):
  embedding gather (indirect DMA, fp8 table, x8 scaled)
  -> PE transpose (regular matmul vs identity, per-sentence 1-bank PSUM)
  -> conv: per fc 5 DRM passes (E[0:256]) + 2 DRM remainder passes
     (E[256:300] paired across shifts w via overlap-stride rhs APs)
     + 1 plain pass (w4) -> tanh (scale 1/64)
  -> attn over tokens: single-op exp + DVE grouped segment reduces
  -> xp projection per 4-position block; LSTM steps interleaved with conv
     (sigmoid via 0.5*tanh(x/2)+0.5 with pre-halved i,f,o weights:
      single ACT table set for the whole kernel)
  -> masked sentence attention + sigmoid output (tanh trick).

Data-parallel over docs: core k handles docs 4k..4k+3; host concatenates.
"""
import sys

for _p in ("/opt/trn_rl_repo", "/root/.axon_site/_ro/trn_rl_repo"):
    if _p not in sys.path:
        sys.path.insert(0, _p)

from contextlib import ExitStack

import numpy as np
import ml_dtypes

import bass_rust
import concourse.bass as bass
from concourse import mybir
from concourse.bass import IndirectOffsetOnAxis
from concourse.bass_utils import run_bass_kernel_spmd
from concourse.masks import make_identity
from concourse.tile import TileContext

from concourse.vector_clock import ScopedClock


class _TC(TileContext):
    """TileContext limiting every instruction to one sem wait (walrus
    rejects multiple); extra waits spill onto same-engine nops."""

    def _commit_instruction(self, inst, lazy_reg_writes: bool = True):
        si = getattr(inst, "sync_info", None)
        if (
            si is not None
            and si.on_wait
            and len(si.on_wait) > 1
            and inst.engine != mybir.EngineType.Unassigned
        ):
            waits = list(si.on_wait)
            inst.sync_info = mybir.SyncInfo(
                on_wait=[waits[-1]], on_update=list(si.on_update or []))
            eng = self.nc.engines[inst.engine]
            for w in waits[:-1]:
                nop = eng.nop().ins
                nop.sync_info = mybir.SyncInfo(on_wait=[w], on_update=[])
        return super()._commit_instruction(inst, lazy_reg_writes)

    def _drain_and_barrier(self, tick_clock, wait_clock):
        carrier = self.nc.sync.nop().ins
        wait_clock.add_sem_waits(
            carrier, ScopedClock({None: tick_clock.global_clock}))
        si = carrier.sync_info
        if si is not None and si.on_wait and len(si.on_wait) > 1:
            waits = list(si.on_wait)
            carrier.sync_info = mybir.SyncInfo(
                on_wait=[waits[0]], on_update=list(si.on_update or []))
            for w in waits[1:]:
                n2 = self.nc.sync.nop().ins
                n2.sync_info = mybir.SyncInfo(on_wait=[w], on_update=[])
        self.nc.sync.drain()
        self.nc.all_engine_barrier()
        assert self.sems is not None
        popped = self.nc._tile_sem_poison_stack.pop()
        assert popped is self._sem_poison
        self.nc.clear_and_free_semaphores(list(self.sems.allocated().values()))
        self.nc.all_engine_barrier()


BF16 = mybir.dt.bfloat16
FP32 = mybir.dt.float32
FP8 = mybir.dt.float8e4
INT32 = mybir.dt.int32
DRM = mybir.MatmulPerfMode.DoubleRow
AF = mybir.ActivationFunctionType
OP = mybir.AluOpType

B, S, L = 32, 64, 128
TOTAL = 1024
V, E, F, W, H = 30000, 300, 256, 5, 256
T = L - W + 1            # 124 valid conv positions
NCORES = 8
DPC = B // NCORES        # 4 docs per core
bf16 = ml_dtypes.bfloat16
f8 = ml_dtypes.float8_e4m3
EMB_SCALE = 8.0          # host pre-scale on Wemb (fp8 denormal avoidance)
WCV_SCALE = 8.0          # host pre-scale on Wconv
CONV_DESCALE = 1.0 / (EMB_SCALE * WCV_SCALE)
WA0_SCALE = 8.0
LSTM_SCALE = 8.0         # host pre-scale on Wih/Whh/bihhh (fp8 denormals)
WA1_SCALE = 8.0


def _ov_ap(base, dims):
    """Copy of `base` AP with raw (stride, count) dims replaced (offset and
    tensor kept). Used for overlap-stride DRM remainder pairs."""
    c = base.copy()
    c.ap = bass_rust.VecI64Pair([tuple(d) for d in dims])
    return c


def build_nc(S_eff: int, group_valid, zero_ba0: bool):
    """group_valid[p] = tuple of doc-slots d (0..3) valid at position p."""
    NP = S_eff                     # number of groups (= positions)
    NLOC = NP * 4
    S4 = S_eff * DPC
    NB = (NP + 3) // 4             # 4-position blocks

    nc = bass.Bass()
    # ---- DRAM I/O ----
    d_idx = nc.dram_tensor("idx_t", [128, NLOC], INT32, kind="ExternalInput")
    d_wemb = nc.dram_tensor("wemb", [V, E], FP8, kind="ExternalInput")
    # conv main: [128K, w(5), sub(2), fc, 128M] fp8 (sub = E 128-chunk)
    d_wcm = nc.dram_tensor("wcm", [128, W, 2, 2, 128], FP8, kind="ExternalInput")
    # remainder pairs (w0,w1),(w2,w3): [44K, pair(2), sub(2), fc, 128M]
    d_wcr = nc.dram_tensor("wcr", [44, 2, 2, 2, 128], FP8, kind="ExternalInput")
    # w4 remainder: [44K, fc, 128M]
    d_wc4 = nc.dram_tensor("wc4", [44, 2, 128], FP8, kind="ExternalInput")
    d_bconv = nc.dram_tensor("bconv_t", [128, 2], FP32, kind="ExternalInput")
    d_wa0 = nc.dram_tensor("wa0_t", [128, 2, 2, 128], FP8, kind="ExternalInput")
    d_ba0 = nc.dram_tensor("ba0_t", [128, 2], FP32, kind="ExternalInput")
    d_wih = nc.dram_tensor("wih_t", [128, 2, 8, 128], FP8, kind="ExternalInput")
    d_bihhh = nc.dram_tensor("bihhh_t", [128, 8, 16], FP32, kind="ExternalInput")
    d_whh = nc.dram_tensor("whh_t", [128, 2, 8, 128], FP8, kind="ExternalInput")
    d_wa1 = nc.dram_tensor("wa1_t", [128, 2, 2, 128], FP8, kind="ExternalInput")
    d_ba1 = nc.dram_tensor("ba1_t", [128, 2], FP32, kind="ExternalInput")
    d_mb1 = nc.dram_tensor("mb1", [1, S4], FP32, kind="ExternalInput")
    d_wo = nc.dram_tensor("wo_t", [128, 2], BF16, kind="ExternalInput")
    d_boh = nc.dram_tensor("bo_half", [1, 1], FP32, kind="ExternalInput")
    d_out = nc.dram_tensor("out", [1, DPC], FP32, kind="ExternalOutput")

    with _TC(nc) as tc, ExitStack() as ctx:
        consts = ctx.enter_context(tc.tile_pool(name="consts", bufs=1))

        ident = consts.tile([128, 128], FP8)
        make_identity(nc, ident[:, :])
        idx_sb = consts.tile([128, NLOC], INT32)
        nc.sync.dma_start(out=idx_sb[:, :], in_=d_idx[:, :])
        wcm_sb = consts.tile([128, W, 2, 2, 128], FP8)
        nc.sync.dma_start(out=wcm_sb[:, :, :, :, :], in_=d_wcm[:, :, :, :, :])
        wcr_sb = consts.tile([44, 2, 2, 2, 128], FP8)
        nc.sync.dma_start(out=wcr_sb[:, :, :, :, :], in_=d_wcr[:, :, :, :, :])
        wc4_sb = consts.tile([44, 2, 128], FP8)
        nc.sync.dma_start(out=wc4_sb[:, :, :], in_=d_wc4[:, :, :])
        bconv_sb = consts.tile([128, 2], FP32)
        nc.sync.dma_start(out=bconv_sb[:, :], in_=d_bconv[:, :])
        wa0_sb = consts.tile([128, 2, 2, 128], FP8)
        nc.sync.dma_start(out=wa0_sb[:, :, :, :], in_=d_wa0[:, :, :, :])
        ba0_sb = consts.tile([128, 2], FP32)
        nc.sync.dma_start(out=ba0_sb[:, :], in_=d_ba0[:, :])
        wih_sb = consts.tile([128, 2, 8, 128], FP8)
        nc.sync.dma_start(out=wih_sb[:, :, :, :], in_=d_wih[:, :, :, :])
        bihhh_sb = consts.tile([128, 8, 16], FP32)
        nc.sync.dma_start(out=bihhh_sb[:, :, :], in_=d_bihhh[:, :, :])
        whh_sb = consts.tile([128, 2, 8, 128], FP8)
        nc.sync.dma_start(out=whh_sb[:, :, :, :], in_=d_whh[:, :, :, :])
        wa1_sb = consts.tile([128, 2, 2, 128], FP8)
        nc.sync.dma_start(out=wa1_sb[:, :, :, :], in_=d_wa1[:, :, :, :])
        ba1_sb = consts.tile([128, 2], FP32)
        nc.sync.dma_start(out=ba1_sb[:, :], in_=d_ba1[:, :])
        mb1_sb = consts.tile([128, S4], FP32)
        nc.sync.dma_start(
            out=mb1_sb[:, :], in_=d_mb1[:, :].to_broadcast([128, S4]))
        wo_sb = consts.tile([128, 2], BF16)
        nc.sync.dma_start(out=wo_sb[:, :], in_=d_wo[:, :])
        boh_sb = consts.tile([1, 1], FP32)
        nc.sync.dma_start(out=boh_sb[:, :], in_=d_boh[:, :])

        # persistent state
        s0T_sb = consts.tile([128, 2, S_eff, DPC], FP8)
        nc.vector.memset(s0T_sb[:, :, :, :], 0.0)
        hsb_sb = consts.tile([128, 2, S_eff, DPC], FP8)
        c_sb = consts.tile([128, 8], FP32)
        nc.vector.memset(c_sb[:, :], 0.0)
        xpt_sb = consts.tile([128, 2, 8, 16], FP32)  # double-buffered blocks

        with (
            tc.tile_pool(name="emb", bufs=3) as p_emb,
            tc.tile_pool(name="ebts", bufs=2) as p_ebts,
            tc.tile_pool(name="c0s", bufs=2) as p_c0,
            tc.tile_pool(name="acts", bufs=2) as p_acts,
            tc.tile_pool(name="small", bufs=2) as p_small,
            tc.tile_pool(name="gsb", bufs=2) as p_gsb,
            tc.tile_pool(name="trp", bufs=2, space="PSUM") as p_trp,
            tc.tile_pool(name="mmp", bufs=1, space="PSUM") as p_mmp,
            tc.tile_pool(name="xpp", bufs=1, space="PSUM") as p_xpp,
            tc.tile_pool(name="gpp", bufs=1, space="PSUM") as p_gpp,
        ):
            def do_gather(b):
                """Gather embeddings for block b -> emb tile [128,16,E]."""
                g0 = 4 * b
                ns = min(4, NP - g0) * 4
                emb_b = p_emb.tile([128, 16, E], FP8, tag="emb")
                for j in range(ns):
                    nc.gpsimd.indirect_dma_start(
                        out=emb_b[:, j, :],
                        out_offset=None,
                        in_=d_wemb[:, :],
                        in_offset=IndirectOffsetOnAxis(
                            ap=idx_sb[:, 4 * g0 + j:4 * g0 + j + 1], axis=0),
                    )
                return emb_b

            def do_transpose(b, emb_b):
                """PE-transpose block b embeddings -> ebt fp8 [128,4,3,512]."""
                g0 = 4 * b
                ng = min(4, NP - g0)
                ebt_b = p_ebts.tile([128, 4, 3, 512], FP8, tag="ebt")
                for gi in range(ng):
                    for s in range(4):
                        tr_ps = p_trp.tile([128, 3, 128], FP32, tag="tr")
                        for ec in range(3):
                            ecw = 128 if ec < 2 else 44
                            nc.tensor.matmul(
                                out=tr_ps[:ecw, ec, :],
                                lhsT=emb_b[:, 4 * gi + s,
                                           128 * ec:128 * ec + ecw],
                                rhs=ident[:, :],
                                start=True, stop=True,
                            )
                        nc.vector.tensor_copy(
                            out=ebt_b[:, gi, 0:2, 128 * s:128 * (s + 1)],
                            in_=tr_ps[:, 0:2, :])
                        nc.scalar.copy(
                            out=ebt_b[0:44, gi, 2, 128 * s:128 * (s + 1)],
                            in_=tr_ps[0:44, 2, :])
                return ebt_b

            def do_conv_block(b, ebt_b):
                """fc-major weight-amortized DRM conv for block b -> c0."""
                g0 = 4 * b
                ng = min(4, NP - g0)
                c0_b = p_c0.tile([128, 4, 2, 512], FP8, tag="c0")
                nc.vector.memset(c0_b[:, :, :, 508:512], 0.0)
                rem2 = p_c0.tile([44, 4, 4, 512], FP8, tag="rem2")
                for gi in range(ng):
                    for j in range(4):
                        nc.vector.tensor_copy(
                            out=rem2[:, gi, j, 0:508],
                            in_=ebt_b[0:44, gi, 2, j:j + 508])
                for fc in range(2):
                    cps = [p_mmp.tile([128, 512], FP32, tag=f"mm{gi}",
                                      name=f"cps{gi}")
                           for gi in range(ng)]
                    # 5 main DRM passes: K = E[0:256] at shift w
                    for w in range(W):
                        for gi in range(ng):
                            nc.tensor.matmul(
                                out=cps[gi][:, 0:508],
                                lhsT=wcm_sb[:, w, :, fc, :],
                                rhs=ebt_b[:, gi, 0:2, w:w + 508],
                                start=(w == 0), stop=False,
                                perf_mode=DRM,
                            )
                    # remainder: 2 DRM pair passes + plain w4, K=44
                    for pr in range(2):
                        for gi in range(ng):
                            nc.tensor.matmul(
                                out=cps[gi][:, 0:508],
                                lhsT=wcr_sb[:, pr, :, fc, :],
                                rhs=rem2[:, gi, 2 * pr:2 * pr + 2, 0:508],
                                start=False, stop=False,
                                perf_mode=DRM,
                            )
                    for gi in range(ng):
                        nc.tensor.matmul(
                            out=cps[gi][:, 0:508],
                            lhsT=wc4_sb[:, fc, :],
                            rhs=ebt_b[0:44, gi, 2, 4:512],
                            start=False, stop=True,
                        )
                    for gi in range(ng):
                        nc.scalar.activation(
                            out=c0_b[:, gi, fc, 0:508],
                            in_=cps[gi][:, 0:508],
                            func=AF.Tanh, scale=CONV_DESCALE,
                            bias=bconv_sb[:, fc:fc + 1])
                return c0_b

            def do_attn0(b, c0_b):
                """logits+softmax+weighted sum for block b -> s0T writes."""
                g0 = 4 * b
                ng = min(4, NP - g0)
                lg = {}
                for mc in range(2):
                    for gi in range(ng):
                        if mc == 0:
                            lg[gi] = p_acts.tile([128, 2, 512], BF16,
                                                 tag=f"lg{gi}", name=f"lg{gi}")
                        lp = p_mmp.tile([128, 512], FP32, tag=f"mm{gi}",
                                        name=f"lp{gi}")
                        nc.tensor.matmul(
                            out=lp[:, 0:508],
                            lhsT=wa0_sb[:, :, mc, :],
                            rhs=c0_b[:, gi, :, 0:508],
                            start=True, stop=True,
                            perf_mode=DRM,
                        )
                        if zero_ba0:
                            nc.scalar.activation(
                                out=lg[gi][:, mc, 0:508], in_=lp[:, 0:508],
                                func=AF.Tanh, scale=1.0 / WA0_SCALE)
                        else:
                            nc.scalar.activation(
                                out=lg[gi][:, mc, 0:508], in_=lp[:, 0:508],
                                func=AF.Tanh, scale=1.0 / WA0_SCALE,
                                bias=ba0_sb[:, mc:mc + 1])
                for gi in range(ng):
                    p = g0 + gi
                    # poison inter-sentence garbage cols (t=124..127) so exp
                    # yields exact zeros there -> full-window reduces below
                    lgv = lg[gi].rearrange("p m (s t) -> p m s t", t=128)
                    nc.vector.memset(lgv[:, :, :, T:128], -1e9)
                    ex = p_acts.tile([128, 2, 512], BF16, tag="ex")
                    nc.scalar.activation(
                        out=ex[:, :, :], in_=lg[gi][:, :, :], func=AF.Exp)
                    prod = p_acts.tile([128, 2, 512], BF16, tag="prod")
                    nc.vector.tensor_tensor(
                        out=prod[:, :, :], in0=ex[:, :, :],
                        in1=c0_b[:, gi, :, :], op=OP.mult)
                    nd = p_small.tile([128, 2, 2, 4], FP32, tag="nd")
                    exv = ex.rearrange("p m (s t) -> p m s t", t=128)
                    prv = prod.rearrange("p m (s t) -> p m s t", t=128)
                    nc.vector.tensor_reduce(
                        out=nd[:, 0, :, :], in_=exv[:, :, :, :],
                        axis=mybir.AxisListType.X, op=OP.add)
                    nc.vector.tensor_reduce(
                        out=nd[:, 1, :, :], in_=prv[:, :, :, :],
                        axis=mybir.AxisListType.X, op=OP.add)
                    nc.vector.reciprocal(
                        out=nd[:, 0, :, :], in_=nd[:, 0, :, :])
                    vd = group_valid[p]
                    if len(vd) == 4:
                        nc.vector.tensor_tensor(
                            out=s0T_sb[:, :, p, :],
                            in0=nd[:, 1, :, :], in1=nd[:, 0, :, :],
                            op=OP.mult)
                    else:
                        for d in vd:
                            nc.vector.tensor_tensor(
                                out=s0T_sb[:, :, p, d:d + 1],
                                in0=nd[:, 1, :, d:d + 1],
                                in1=nd[:, 0, :, d:d + 1], op=OP.mult)

            def do_xp(b):
                """x-projection for block b positions 4b..4b+3 (+bias)."""
                n = min(4, NP - 4 * b) * DPC
                xps = p_xpp.tile([128, 8, 16], FP32, tag="xp")
                for gt in range(8):
                    nc.tensor.matmul(
                        out=xps[:, gt, 0:n],
                        lhsT=wih_sb[:, :, gt, :],
                        rhs=s0T_sb[:, :, 4 * b:4 * b + (n // DPC), :],
                        start=True, stop=True,
                        perf_mode=DRM,
                    )
                nc.vector.tensor_tensor(
                    out=xpt_sb[:, b % 2, :, 0:n],
                    in0=xps[:, :, 0:n], in1=bihhh_sb[:, :, 0:n], op=OP.add)

            def do_lstm_step(t):
                """one LSTM step; gate order [i,f,o,g]; i,f,o pre-halved."""
                blk, q = t // 4, t % 4
                ga = p_gsb.tile([128, 8, 4], FP32, tag="ga")
                if t == 0:
                    nc.scalar.activation(
                        out=ga[:, :, :],
                        in_=xpt_sb[:, blk % 2, :, 4 * q:4 * q + 4],
                        func=AF.Tanh, scale=1.0 / LSTM_SCALE)
                else:
                    gps = p_gpp.tile([128, 8, 4], FP32, tag="g")
                    for gt in range(8):
                        nc.tensor.matmul(
                            out=gps[:, gt, :],
                            lhsT=whh_sb[:, :, gt, :],
                            rhs=hsb_sb[:, :, t - 1, :],
                            start=True, stop=True,
                            perf_mode=DRM,
                        )
                    sm = p_gsb.tile([128, 8, 4], FP32, tag="sm")
                    nc.vector.tensor_tensor(
                        out=sm[:, :, :], in0=gps[:, :, :],
                        in1=xpt_sb[:, blk % 2, :, 4 * q:4 * q + 4],
                        op=OP.add)
                    nc.scalar.activation(
                        out=ga[:, :, :], in_=sm[:, :, :], func=AF.Tanh,
                        scale=1.0 / LSTM_SCALE)
                gaf = ga.rearrange("p g d -> p (g d)")
                nc.vector.tensor_scalar(
                    out=gaf[:, 0:24], in0=gaf[:, 0:24],
                    scalar1=0.5, scalar2=0.5, op0=OP.mult, op1=OP.add)
                tmp = p_gsb.tile([128, 8], FP32, tag="tmp")
                nc.vector.tensor_tensor(
                    out=tmp[:, :], in0=gaf[:, 0:8], in1=gaf[:, 24:32],
                    op=OP.mult)
                nc.vector.tensor_tensor(
                    out=c_sb[:, :], in0=c_sb[:, :], in1=gaf[:, 8:16],
                    op=OP.mult)
                nc.vector.tensor_tensor(
                    out=c_sb[:, :], in0=c_sb[:, :], in1=tmp[:, :], op=OP.add)
                tch = p_gsb.tile([128, 8], FP32, tag="tch")
                nc.scalar.activation(out=tch[:, :], in_=c_sb[:, :],
                                     func=AF.Tanh)
                nc.vector.tensor_tensor(
                    out=hsb_sb[:, :, t, :],
                    in0=gaf[:, 16:24].rearrange("p (k d) -> p k d", d=DPC),
                    in1=tch.rearrange("p (k d) -> p k d", d=DPC), op=OP.mult)

            # ---------------- pipeline over blocks ----------------
            state = {"step": 0}

            def emit_steps(upto):
                while state["step"] < min(upto, NP):
                    do_lstm_step(state["step"])
                    state["step"] += 1

            embs = {}
            ebts = {}
            embs[0] = do_gather(0)
            if NB > 1:
                embs[1] = do_gather(1)
            ebts[0] = do_transpose(0, embs[0])
            for b in range(NB):
                if b + 2 < NB:
                    embs[b + 2] = do_gather(b + 2)
                if b + 1 < NB:
                    ebts[b + 1] = do_transpose(b + 1, embs[b + 1])
                    embs.pop(b, None)
                emit_steps(4 * (b - 1) - 1)
                c0_b = do_conv_block(b, ebts[b])
                emit_steps(4 * (b - 1))
                do_attn0(b, c0_b)
                emit_steps(4 * (b - 1) + 1)
                do_xp(b)
                emit_steps(4 * (b - 1) + 2)
                ebts.pop(b, None)
            emit_steps(NP)

            # ================= sentence attention + output ============
            l1_sb = p_acts.tile([128, 2, S4], FP32, tag="l1f")
            for mc in range(2):
                l1_ps = p_mmp.tile([128, S4], FP32, tag="mm0", name="l1ps")
                nc.tensor.matmul(
                    out=l1_ps[:, :],
                    lhsT=wa1_sb[:, :, mc, :],
                    rhs=hsb_sb[:, :, :, :],
                    start=True, stop=True,
                    perf_mode=DRM,
                )
                nc.scalar.activation(
                    out=l1_sb[:, mc, :], in_=l1_ps[:, :],
                    func=AF.Tanh, scale=1.0 / WA1_SCALE,
                    bias=ba1_sb[:, mc:mc + 1])
                nc.vector.tensor_tensor(
                    out=l1_sb[:, mc, :], in0=l1_sb[:, mc, :],
                    in1=mb1_sb[:, :], op=OP.add)
            ex1 = p_acts.tile([128, 2, S4], FP32, tag="ex1")
            nc.scalar.activation(
                out=ex1[:, :, :], in_=l1_sb[:, :, :], func=AF.Exp)
            prod1 = p_acts.tile([128, 2, S4], FP32, tag="prod1")
            h1v = hsb_sb[:, :, :, :]
            nc.vector.tensor_tensor(
                out=prod1.rearrange("p m (t d) -> p m t d", d=DPC),
                in0=ex1.rearrange("p m (t d) -> p m t d", d=DPC),
                in1=h1v, op=OP.mult)
            nd1 = p_small.tile([128, 2, 2, DPC], FP32, tag="nd1")
            e1v = ex1.rearrange("p m (t d) -> p m t d", d=DPC)
            p1v = prod1.rearrange("p m (t d) -> p m t d", d=DPC)
            nc.vector.tensor_reduce(
                out=nd1[:, 0, :, :], in_=e1v.transpose([0, 1, 3, 2]),
                axis=mybir.AxisListType.X, op=OP.add)
            nc.vector.tensor_reduce(
                out=nd1[:, 1, :, :], in_=p1v.transpose([0, 1, 3, 2]),
                axis=mybir.AxisListType.X, op=OP.add)
            nc.vector.reciprocal(out=nd1[:, 0, :, :], in_=nd1[:, 0, :, :])
            s1_sb = p_small.tile([128, 2, DPC], BF16, tag="s1")
            nc.vector.tensor_tensor(
                out=s1_sb[:, :, :], in0=nd1[:, 1, :, :],
                in1=nd1[:, 0, :, :], op=OP.mult)
            o_ps = p_xpp.tile([128, DPC], FP32, tag="xp", name="ops")
            for kc in range(2):
                nc.tensor.matmul(
                    out=o_ps[:1, :],
                    lhsT=wo_sb[:, kc:kc + 1],
                    rhs=s1_sb[:, kc, :],
                    start=(kc == 0), stop=(kc == 1),
                )
            y_sb = p_small.tile([1, DPC], FP32, tag="y")
            nc.scalar.activation(
                out=y_sb[:, :], in_=o_ps[:1, :],
                func=AF.Tanh, bias=boh_sb[:1, :1], scale=0.5)
            nc.vector.tensor_scalar(
                out=y_sb[:, :], in0=y_sb[:, :],
                scalar1=0.5, scalar2=0.5, op0=OP.mult, op1=OP.add)
            nc.sync.dma_start(out=d_out[:, :], in_=y_sb[:, :])

    return nc


def _host_prep(inputs):
    inp = {k: np.asarray(v) for k, v in inputs.items()}
    tok = inp["input"].astype(np.int32)
    num_sent = inp["num_sent"].astype(np.int64)
    mask = np.asarray(inp["mask"], np.float32)

    S_eff = max(int(num_sent.max()), 1)
    batch_ids = np.repeat(np.arange(B), num_sent)
    if len(batch_ids) < TOTAL:
        batch_ids = np.concatenate(
            [batch_ids, np.full(TOTAL - len(batch_ids), B - 1, np.int64)])
    batch_ids = batch_ids[:TOTAL]
    offsets = np.cumsum(num_sent) - num_sent
    pos = np.arange(TOTAL) - offsets[batch_ids]
    valid = pos < num_sent[batch_ids]

    # per-core, position-major: slot (p, d) -> global sentence id or -1
    slot = -np.ones((NCORES, S_eff, DPC), np.int64)
    for j in range(TOTAL):
        if valid[j]:
            doc = int(batch_ids[j])
            slot[doc // DPC, int(pos[j]), doc % DPC] = j
    group_valid = tuple(
        tuple(d for d in range(DPC) if slot[0, p, d] >= 0)
        for p in range(S_eff))
    uniform = all(
        (slot[k, p, d] >= 0) == (slot[0, p, d] >= 0)
        for k in range(NCORES) for p in range(S_eff) for d in range(DPC))

    zero_ba0 = not np.any(np.asarray(inp["ba0"], np.float32))

    wemb = (np.asarray(inp["Wemb"], np.float32) * EMB_SCALE).astype(f8)
    wc = np.asarray(inp["Wconv"], np.float32) * WCV_SCALE  # [F,1,W,E]
    wcm = np.zeros((128, W, 2, 2, 128), f8)
    wcr = np.zeros((44, 2, 2, 2, 128), f8)
    wc4 = np.zeros((44, 2, 128), f8)
    for fc in range(2):
        blk = wc[128 * fc:128 * (fc + 1), 0, :, :]  # [128M, W, E]
        for w in range(W):
            for sub in range(2):
                wcm[:, w, sub, fc, :] = (
                    blk[:, w, 128 * sub:128 * (sub + 1)].T.astype(f8))
        for pr in range(2):
            for sub in range(2):
                wcr[:, pr, sub, fc, :] = (
                    blk[:, 2 * pr + sub, 256:300].T.astype(f8))
        wc4[:, fc, :] = blk[:, 4, 256:300].T.astype(f8)
    bconv_t = np.asarray(inp["bconv"], np.float32).reshape(2, 128).T.copy()
    wa0_t = _pack_kx(np.asarray(inp["Wa0"], np.float32) * WA0_SCALE, f8)
    ba0_t = np.asarray(inp["ba0"], np.float32).reshape(2, 128).T.copy()
    _gperm = np.concatenate([np.arange(0, 256), np.arange(256, 512),
                             np.arange(768, 1024), np.arange(512, 768)])
    # pre-halve i,f,o rows (first 768 after perm) for sigmoid-as-tanh
    _gscale = np.concatenate([np.full(768, 0.5), np.ones(256)])
    _gscale = _gscale * LSTM_SCALE
    wih_t = _pack_kx(
        (np.asarray(inp["Wih"], np.float32)[_gperm]
         * _gscale[:, None]).T, f8)
    whh_t = _pack_kx(
        (np.asarray(inp["Whh"], np.float32)[_gperm]
         * _gscale[:, None]).T, f8)
    bihhh_t = (((np.asarray(inp["bih"], np.float32)
                 + np.asarray(inp["bhh"], np.float32))[_gperm] * _gscale)
               ).reshape(8, 128).T.copy()
    bihhh_t = np.repeat(bihhh_t[:, :, None], 16, axis=2).copy()
    wa1_t = _pack_kx(np.asarray(inp["Wa1"], np.float32) * WA1_SCALE, f8)
    ba1_t = np.asarray(inp["ba1"], np.float32).reshape(2, 128).T.copy()
    wo_t = np.asarray(inp["Wo"], np.float32).reshape(2, 128).T.astype(bf16).copy()
    bo_half = (0.5 * np.asarray(inp["bo"], np.float32)).reshape(1, 1)

    in_maps = []
    for k in range(NCORES):
        idx_t = np.zeros((128, S_eff * 4), np.int32)
        for p in range(S_eff):
            for d in range(DPC):
                sj = slot[k, p, d]
                if sj >= 0:
                    idx_t[:, 4 * p + d] = tok[sj]
        mb1 = np.zeros((1, S_eff * DPC), np.float32)
        for d in range(DPC):
            doc = k * DPC + d
            mvals = mask[doc, :S_eff, 0]
            mb1[0, np.arange(S_eff) * DPC + d] = np.where(mvals > 0, 0.0, -1e9)
        in_maps.append({
            "idx_t": idx_t, "wemb": wemb, "wcm": wcm, "wcr": wcr,
            "wc4": wc4, "bconv_t": bconv_t, "wa0_t": wa0_t, "ba0_t": ba0_t,
            "wih_t": wih_t, "bihhh_t": bihhh_t, "whh_t": whh_t,
            "wa1_t": wa1_t, "ba1_t": ba1_t, "mb1": mb1,
            "wo_t": wo_t, "bo_half": bo_half,
        })
    return S_eff, group_valid, uniform, zero_ba0, in_maps


def _pack_kx(w, dt=bf16):
    """[K=256, M_total] -> [128, kc, mt, 128] tile pack."""
    w = np.asarray(w, np.float32)
    K, M = w.shape
    assert K == 256 and M % 128 == 0
    mt = M // 128
    out = np.zeros((128, 2, mt, 128), dt)
    for kc in range(2):
        for m in range(mt):
            out[:, kc, m, :] = w[128 * kc:128 * (kc + 1),
                                 128 * m:128 * (m + 1)].astype(dt)
    return out


_NC_CACHE = {}


def kernel(**inputs) -> np.ndarray:
    S_eff, group_valid, uniform, zba0, in_maps = _host_prep(inputs)
    assert uniform, "non-uniform sentence layout not supported"
    out = np.zeros((B, 1), np.float32)
    key = (S_eff, group_valid, zba0)
    if key not in _NC_CACHE:
        _NC_CACHE[key] = build_nc(S_eff, group_valid, zba0)
    nc = _NC_CACHE[key]
    res = run_bass_kernel_spmd(nc, in_maps, core_ids=list(range(NCORES)))
    for k in range(NCORES):
        out[k * DPC:(k + 1) * DPC, 0] = res.results[k]["out"][0]
    return out
